# revision 11
# baseline (speedup 1.0000x reference)
"""Trainium2 Bass kernel for nn_Attention_77824807403911 (sparse_attention).

Math (per batch element, no softmax => associativity):
    q = x @ Wq^T + bq ; v = x @ Wv^T + bv          [1024, 256]
    rq = rope(q) ; rv = rope(v)
    per head h (16 heads, hd=16):  att_h = rq_h @ (rq_h^T @ rv_h) / 4
    out = att @ Wo^T + bo

Instead of the 1024x1024 score matrix we compute the 16x16 Gram per head
(64x fewer flops), realized as a full 256x256 Gram masked to the
block-diagonal, folded with Wo into a single per-batch [256,256] weight:
    F[e,f]  = sum_s rv[s,e] rq[s,f]       (Gram, transposed blocks)
    BDT     = F .* blockmask
    W2[f,o] = sum_e BDT[e,f] * Wo[o,e] / 4
    outT    = W2^T @ rqT + bo             ([256, 1024])

Sharding: data-parallel over batch, 1 element per core, no collectives.

Schedule (v3): inputs split into ordered small DMAs on both HWDGE
rings; PE warm-up ramp; s-chunked (512) pipeline proj (PE) -> evict+
bias (ACT) -> rope (DVE + 4 ops on GpSimd) -> xbar transpose (Sync/
Scalar) -> Gram accumulate (PE); bf16 output (host upcasts).
"""

import numpy as np
import ml_dtypes

import concourse.bass as bass
import concourse.bacc as bacc
import concourse.tile as tile
from concourse import mybir
from concourse.bass_utils import run_bass_kernel_spmd

B, S, D, H, HD = 8, 1024, 256, 16, 16
N_CORES = 8
BF16 = mybir.dt.bfloat16
F32 = mybir.dt.float32

# channel permutation: [evens of pairs 0-63 (theta=1), evens of pairs 64-127
# (theta=1e-4), odds of pairs 0-63, odds of pairs 64-127]
PERM = np.concatenate(
    [np.arange(0, 128, 2), np.arange(128, 256, 2),
     np.arange(1, 128, 2), np.arange(129, 256, 2)]
)


def _host_tables():
    p = np.arange(128)
    theta = np.where(p < 64, 1.0, 1e-4)
    s = np.arange(S, dtype=np.float64) + 1.0
    ang = theta[:, None] * s[None, :]
    sin = np.sin(ang).astype(ml_dtypes.bfloat16)
    cos = np.cos(ang).astype(ml_dtypes.bfloat16)
    a = np.arange(256)
    headp = (a % 128) // 8
    mask = (headp[:, None] == headp[None, :]).astype(ml_dtypes.bfloat16)
    return sin, cos, mask


def build_kernel():
    nc = bacc.Bacc()
    xT = nc.declare_dram_parameter("xT", [D, S], BF16, isOutput=False)
    # wbig columns: [wqt | wvt | wot | mask], each [256, 256]; then 3 bias cols
    wbig = nc.declare_dram_parameter("wbig", [D, 4 * D + 3], BF16, isOutput=False)
    # trig columns: [sin | cos], each [128, 1024]
    trig = nc.declare_dram_parameter("trig", [128, 2 * S], BF16, isOutput=False)
    outT = nc.declare_dram_parameter("outT", [D, S], BF16, isOutput=True)

    with tile.TileContext(nc) as tc:
        _body(tc, xT, wbig, trig, outT)
    nc.compile()
    return nc


def _body(tc, xT, wbig, trig, outT):
    nc = tc.nc
    NS = 2          # s chunks of 512
    SC = S // NS    # 512

    with (
        tc.tile_pool(name="const", bufs=1) as cpool,
        tc.tile_pool(name="acts", bufs=1) as apool,
        tc.tile_pool(name="psum", bufs=4, space="PSUM") as pp,
        tc.tile_pool(name="outp", bufs=4) as opool,
    ):
        scratch = cpool.tile([128, 512], BF16, tag="scratch", name="scratch")
        nc.gpsimd.memset(scratch[:], 0.25)

        # PE clock ramp: F=512 garbage matmuls while inputs stream in
        for wi in range(4):
            warm_ps = pp.tile([128, 512], F32, tag="warm", bufs=1,
                              name=f"warm_ps{wi}")
            nc.tensor.matmul(warm_ps[:], scratch[:, 0:128], scratch[:],
                             start=True, stop=True, skip_group_check=True)

        # ---- input DMAs, ordered for earliest first-matmul ----
        trig_sb = cpool.tile([128, 2 * S], BF16, tag="trig", name="trig_sb")
        w_sb = [cpool.tile([128, 4 * D + 3], BF16, tag=f"wbig{cc}",
                           name=f"wbig{cc}") for cc in range(2)]
        xT_sb = [cpool.tile([128, S], BF16, tag=f"xT{cc}", name=f"xT{cc}")
                 for cc in range(2)]
        # sync ring: wbig0, x00, x01, x11, cos ; scalar ring: wbig1, x10, sin
        nc.sync.dma_start(w_sb[0][:], wbig[0:128, :])
        nc.scalar.dma_start(w_sb[1][:], wbig[128:256, :])
        nc.sync.dma_start(xT_sb[0][:, 0:SC], xT[0:128, 0:SC])
        nc.scalar.dma_start(xT_sb[1][:, 0:SC], xT[128:256, 0:SC])
        nc.sync.dma_start(xT_sb[0][:, SC:S], xT[0:128, SC:S])
        nc.sync.dma_start(xT_sb[1][:, SC:S], xT[128:256, SC:S])
        nc.scalar.dma_start(trig_sb[:, 0:S], trig[:, 0:S])
        nc.sync.dma_start(trig_sb[:, S:2 * S], trig[:, S:2 * S])
        sin_sb = trig_sb[:, 0:S]
        cos_sb = trig_sb[:, S:2 * S]

        def wslice(idx, cc, col0, ncol):
            return w_sb[cc][:, idx * D + col0: idx * D + col0 + ncol]

        def bias_ap(idx, cc):
            return w_sb[cc][:, 4 * D + idx: 4 * D + idx + 1]

        def act2(tag, width=S, dtype=BF16):
            return [apool.tile([128, width], dtype, tag=f"{tag}{cc}",
                               name=f"{tag}{cc}") for cc in range(2)]

        qT = act2("qT")
        vT = act2("vT")
        rqT = act2("rqT")
        rvT = act2("rvT")
        rq_nat = [apool.tile([128, 4 * D], BF16, tag=f"rqn{sc}", name=f"rqn{sc}")
                  for sc in range(NS)]
        rv_nat = [apool.tile([128, 4 * D], BF16, tag=f"rvn{sc}", name=f"rvn{sc}")
                  for sc in range(NS)]

        # ---- projections, PE order: q0, v0, q1, v1 ----
        q_ps = [[None, None] for _ in range(NS)]
        v_ps = [[None, None] for _ in range(NS)]

        def proj_chunk(widx, sc, dst_ps):
            for ac in range(2):
                ps = pp.tile([128, SC], F32, tag="mm", bufs=4,
                             name=f"proj_ps_{widx}_{sc}_{ac}")
                for dc in range(2):
                    nc.tensor.matmul(
                        ps[:],
                        wslice(widx, dc, ac * 128, 128),
                        xT_sb[dc][:, sc * SC:(sc + 1) * SC],
                        start=(dc == 0), stop=(dc == 1),
                    )
                dst_ps[ac] = ps

        for sc in range(NS):
            proj_chunk(0, sc, q_ps[sc])
            proj_chunk(1, sc, v_ps[sc])

        # ---- ACT evict stream: qe(sc), ve(sc) interleaved ----
        def evict(ps, bidx, dstT, ac, sc):
            nc.scalar.activation(
                dstT[ac][:, sc * SC:(sc + 1) * SC], ps[:],
                mybir.ActivationFunctionType.Identity, bias=bias_ap(bidx, ac))

        for sc in range(NS):
            evict(q_ps[sc][0], 0, qT, 0, sc)
            evict(q_ps[sc][1], 0, qT, 1, sc)
            evict(v_ps[sc][0], 1, vT, 0, sc)
            evict(v_ps[sc][1], 1, vT, 1, sc)

        # ---- rope: q on DVE; v mults split DVE/GpSimd ----
        def rope_chunk(srcT, dstT, sc, tmp_tag, pool_ops=()):
            sl = slice(sc * SC, (sc + 1) * SC)
            E, O = srcT[0][:, sl], srcT[1][:, sl]
            ssl = sin_sb[:, sl]
            csl = cos_sb[:, sl]
            m1 = opool.tile([128, SC], BF16, tag="m1", bufs=2)
            m2 = opool.tile([128, SC], BF16, tag="m2", bufs=2)
            m3 = opool.tile([128, SC], BF16, tag="m3", bufs=2)
            m4 = opool.tile([128, SC], BF16, tag="m4", bufs=2)
            def eng(name):
                return nc.gpsimd if name in pool_ops else nc.vector
            eng("m1").tensor_tensor(m1[:], E, ssl, mybir.AluOpType.mult)
            eng("m3").tensor_tensor(m3[:], E, csl, mybir.AluOpType.mult)
            eng("m2").tensor_tensor(m2[:], O, csl, mybir.AluOpType.mult)
            eng("m4").tensor_tensor(m4[:], O, ssl, mybir.AluOpType.mult)
            nc.vector.tensor_tensor(dstT[0][:, sl], m1[:], m2[:],
                                    mybir.AluOpType.subtract)
            nc.vector.tensor_tensor(dstT[1][:, sl], m3[:], m4[:],
                                    mybir.AluOpType.add)

        # ---- transposes (xbar); engine chosen per call ----
        def transpose_chunk(srcT, nat, sc, engines):
            nat3 = nat[:].rearrange("p (st c) -> p st c", c=D)
            for cc in range(2):
                engines[cc].dma_start(
                    nat3[:, :, cc * 128:(cc + 1) * 128],
                    srcT[cc][:, sc * SC:(sc + 1) * SC], transpose=True)

        # DVE emission order: q0, v0(partial), q1, v1(partial)
        rope_chunk(qT, rqT, 0, "rq0")
        transpose_chunk(rqT, rq_nat[0], 0, [nc.sync, nc.sync])
        rope_chunk(vT, rvT, 0, "rv0", pool_ops=("m1", "m3"))
        transpose_chunk(rvT, rv_nat[0], 0, [nc.scalar, nc.sync])
        rope_chunk(qT, rqT, 1, "rq1")
        transpose_chunk(rqT, rq_nat[1], 1, [nc.sync, nc.scalar])
        rope_chunk(vT, rvT, 1, "rv1", pool_ops=("m2", "m4"))
        transpose_chunk(rvT, rv_nat[1], 1, [nc.sync, nc.scalar])

        # ---- Gram: Hm[e, f] = sum_s rv[s, e] rq[s, f] (PE, accumulated) ----
        gram_ps = [pp.tile([128, D], F32, tag="sm", bufs=2, name=f"gram_ps{ec}")
                   for ec in range(2)]
        for sc in range(NS):
            for ec in range(2):
                for stl in range(4):
                    st = sc * 4 + stl
                    nc.tensor.matmul(
                        gram_ps[ec][:],
                        rv_nat[sc][:, stl * D + ec * 128: stl * D + (ec + 1) * 128],
                        rq_nat[sc][:, stl * D: (stl + 1) * D],
                        start=(st == 0), stop=(st == 7),
                        skip_group_check=True,
                    )

        # ---- mask -> BDT (DVE) ----
        bdt = act2("bdt", width=D)
        for ec in range(2):
            nc.vector.tensor_tensor(
                bdt[ec][:], gram_ps[ec][:], wslice(3, ec, 0, D),
                mybir.AluOpType.mult)

        # ---- W2[f, o] = sum_e BDT[e, f] wot[e, o] / 4 (evict on ACT) ----
        w2 = act2("w2", width=D)
        for fc in range(2):
            ps = pp.tile([128, D], F32, tag="sm", bufs=2, name=f"w2_ps{fc}")
            for ec in range(2):
                nc.tensor.matmul(
                    ps[:],
                    bdt[ec][:, fc * 128:(fc + 1) * 128],
                    wslice(2, ec, 0, D),
                    start=(ec == 0), stop=(ec == 1),
                )
            nc.scalar.activation(w2[fc][:], ps[:],
                                 mybir.ActivationFunctionType.Copy, scale=0.25)

        # ---- final: outT[o, s] = sum_f W2[f, o] rqT[f, s] + bo ----
        for oc in range(2):
            for sc in range(NS):
                ps = pp.tile([128, SC], F32, tag="mm", bufs=4,
                             name=f"fin_ps{oc}{sc}")
                for fc in range(2):
                    nc.tensor.matmul(
                        ps[:],
                        w2[fc][:, oc * 128:(oc + 1) * 128],
                        rqT[fc][:, sc * SC:(sc + 1) * SC],
                        start=(fc == 0), stop=(fc == 1),
                    )
                ot = opool.tile([128, SC], BF16, tag="out_sb", bufs=4,
                                name=f"out_sb{oc}{sc}")
                nc.scalar.activation(
                    ot[:], ps[:],
                    mybir.ActivationFunctionType.Identity,
                    bias=bias_ap(2, oc),
                )
                nc.sync.dma_start(
                    outT[oc * 128:(oc + 1) * 128, sc * SC:(sc + 1) * SC], ot[:])


_NC_CACHE = None


def _get_nc():
    global _NC_CACHE
    if _NC_CACHE is None:
        _NC_CACHE = build_kernel()
    return _NC_CACHE


def make_in_maps(x, wq_w, wq_b, wv_w, wv_b, wo_w, wo_b):
    sin, cos, mask = _host_tables()
    wq_p = np.ascontiguousarray(wq_w[PERM].T).astype(ml_dtypes.bfloat16)   # [d, a]
    wv_p = np.ascontiguousarray(wv_w[PERM].T).astype(ml_dtypes.bfloat16)
    wo_p = np.ascontiguousarray(wo_w[:, PERM].T).astype(ml_dtypes.bfloat16)  # [a(e), o]
    bias3 = np.stack([wq_b[PERM], wv_b[PERM], wo_b], axis=1).astype(ml_dtypes.bfloat16)
    wbig = np.ascontiguousarray(
        np.concatenate([wq_p, wv_p, wo_p, mask, bias3], axis=1))
    trig = np.ascontiguousarray(np.concatenate([sin, cos], axis=1))
    in_maps = []
    for b in range(B):
        in_maps.append({
            "xT": np.ascontiguousarray(x[b].T).astype(ml_dtypes.bfloat16),
            "wbig": wbig, "trig": trig,
        })
    return in_maps


TRACE = False
RUN_KWARGS = {}
LAST_RESULT = None


def kernel(x, wq_w, wq_b, wk_w, wk_b, wv_w, wv_b, wo_w, wo_b):
    global LAST_RESULT
    x = np.asarray(x, dtype=np.float32)
    in_maps = make_in_maps(x, np.asarray(wq_w, np.float32), np.asarray(wq_b, np.float32),
                           np.asarray(wv_w, np.float32), np.asarray(wv_b, np.float32),
                           np.asarray(wo_w, np.float32), np.asarray(wo_b, np.float32))
    nc = _get_nc()
    res = run_bass_kernel_spmd(nc, in_maps, core_ids=list(range(N_CORES)),
                               trace=TRACE, **RUN_KWARGS)
    LAST_RESULT = res
    outs = [np.ascontiguousarray(res.results[b]["outT"].T) for b in range(B)]
    return np.stack(outs).astype(np.float32)


# revision 13
# speedup vs baseline: 1.0092x; 1.0092x over previous
"""Trainium2 Bass kernel for nn_Attention_77824807403911 (sparse_attention).

Math (per batch element, no softmax => associativity):
    q = x @ Wq^T + bq ; v = x @ Wv^T + bv          [1024, 256]
    rq = rope(q) ; rv = rope(v)
    per head h (16 heads, hd=16):  att_h = rq_h @ (rq_h^T @ rv_h) / 4
    out = att @ Wo^T + bo

Instead of the 1024x1024 score matrix we compute the 16x16 Gram per head
(64x fewer flops), realized as a full 256x256 Gram masked to the
block-diagonal, folded with Wo into a single per-batch [256,256] weight:
    F[e,f]  = sum_s rv[s,e] rq[s,f]       (Gram, transposed blocks)
    BDT     = F .* blockmask
    W2[f,o] = sum_e BDT[e,f] * Wo[o,e] / 4
    outT    = W2^T @ rqT + bo             ([256, 1024])

Sharding: data-parallel over batch, 1 element per core, no collectives.

Schedule (v4):
- input cut to ~0.9MB: trig tables and the block mask are generated
  on-chip via tiny PE broadcast matmuls (rank-2 / rank-16);
- per-ring DMA order puts the first-needed bytes (wq, first x chunks)
  at the ring heads;
- PE clock-ramp warmups sized to end when the first x chunk lands;
- s-chunked (512) pipeline proj (PE) -> evict+bias (ACT) -> rope (all
  DVE; GpSimd poisons concurrent DVE ops via the shared SBUF port) ->
  xbar transpose (Sync/Scalar) -> Gram accumulate (PE, per-slice deps);
- bf16 output on both rings (host upcasts).
"""

import numpy as np
import ml_dtypes

import concourse.bass as bass
import concourse.bacc as bacc
import concourse.tile as tile
from concourse import mybir
from concourse.bass_utils import run_bass_kernel_spmd

B, S, D, H, HD = 8, 1024, 256, 16, 16
N_CORES = 8
BF16 = mybir.dt.bfloat16
F32 = mybir.dt.float32

# channel permutation: [evens of pairs 0-63 (theta=1), evens of pairs 64-127
# (theta=1e-4), odds of pairs 0-63, odds of pairs 64-127]
PERM = np.concatenate(
    [np.arange(0, 128, 2), np.arange(128, 256, 2),
     np.arange(1, 128, 2), np.arange(129, 256, 2)]
)


def _host_tables():
    s = np.arange(S, dtype=np.float64) + 1.0
    trig = np.stack([
        np.sin(s), np.cos(s), np.sin(1e-4 * s), np.cos(1e-4 * s)
    ])                                      # [4, 1024]
    sel_sin = np.zeros((4, 128))
    sel_cos = np.zeros((4, 128))
    sel_sin[0, 0:64] = 1.0
    sel_sin[2, 64:128] = 1.0
    sel_cos[1, 0:64] = 1.0
    sel_cos[3, 64:128] = 1.0
    trigB = np.concatenate([trig, sel_sin, sel_cos],
                           axis=1).astype(ml_dtypes.bfloat16)  # [4, 1280]
    # mask = A^T B, rank 16: A[h,p] = (p//8 == h), B[h,c] = ((c%128)//8 == h)
    h = np.arange(16)
    p = np.arange(128)
    c = np.arange(256)
    A = (p[None, :] // 8 == h[:, None]).astype(np.float64)       # [16,128]
    Bm = ((c[None, :] % 128) // 8 == h[:, None]).astype(np.float64)  # [16,256]
    maskAB = np.concatenate([A, Bm], axis=1).astype(ml_dtypes.bfloat16)  # [16,384]
    return trigB, maskAB


def build_kernel():
    nc = bacc.Bacc()
    xT = nc.declare_dram_parameter("xT", [D, S], BF16, isOutput=False)
    # wbig columns: [wqt | wvt | wot], each [256, 256]; then 3 bias cols
    wbig = nc.declare_dram_parameter("wbig", [D, 3 * D + 3], BF16, isOutput=False)
    # trigB rows: sin(s), cos(s), sin(1e-4 s), cos(1e-4 s), s=1..1024, then
    # sel_sin [4,128], sel_cos [4,128]
    trigB = nc.declare_dram_parameter("trigB", [4, S + 256], BF16, isOutput=False)
    maskAB = nc.declare_dram_parameter("maskAB", [16, 384], BF16, isOutput=False)
    outT = nc.declare_dram_parameter("outT", [D, S], BF16, isOutput=True)

    with tile.TileContext(nc) as tc:
        _body(tc, xT, wbig, trigB, maskAB, outT)
    nc.compile()
    return nc


def _body(tc, xT, wbig, trigB, maskAB, outT):
    nc = tc.nc
    NS = 2          # s chunks of 512
    SC = S // NS    # 512

    with (
        tc.tile_pool(name="const", bufs=1) as cpool,
        tc.tile_pool(name="acts", bufs=1) as apool,
        tc.tile_pool(name="psum", bufs=4, space="PSUM") as pp,
        tc.tile_pool(name="outp", bufs=4) as opool,
    ):
        scratch = cpool.tile([128, 512], BF16, tag="scratch", name="scratch")
        nc.gpsimd.memset(scratch[:], 0.25)

        # ---- input DMAs: ring heads carry first-needed bytes ----
        trigB_sb = cpool.tile([4, S + 256], BF16, tag="trigB", name="trigB_sb")
        maskAB_sb = cpool.tile([16, 384], BF16, tag="maskAB", name="maskAB_sb")
        w_sb = [cpool.tile([128, 3 * D + 3], BF16, tag=f"wbig{cc}",
                           name=f"wbig{cc}") for cc in range(2)]
        xT_sb = [cpool.tile([128, S], BF16, tag=f"xT{cc}", name=f"xT{cc}")
                 for cc in range(2)]
        # sync ring: trigB, wq0, x00, x01, wv0, (wo0+bias)
        nc.sync.dma_start(trigB_sb[:], trigB[:])
        nc.sync.dma_start(w_sb[0][:, 0:D], wbig[0:128, 0:D])
        nc.sync.dma_start(xT_sb[0][:, 0:SC], xT[0:128, 0:SC])
        nc.sync.dma_start(xT_sb[0][:, SC:S], xT[0:128, SC:S])
        nc.sync.dma_start(w_sb[0][:, D:2 * D], wbig[0:128, D:2 * D])
        nc.sync.dma_start(w_sb[0][:, 2 * D:3 * D + 3], wbig[0:128, 2 * D:3 * D + 3])
        # scalar ring: maskAB, wq1, x10, x11, wv1, (wo1+bias)
        nc.scalar.dma_start(maskAB_sb[:], maskAB[:])
        nc.scalar.dma_start(w_sb[1][:, 0:D], wbig[128:256, 0:D])
        nc.scalar.dma_start(xT_sb[1][:, 0:SC], xT[128:256, 0:SC])
        nc.scalar.dma_start(xT_sb[1][:, SC:S], xT[128:256, SC:S])
        nc.scalar.dma_start(w_sb[1][:, D:2 * D], wbig[128:256, D:2 * D])
        nc.scalar.dma_start(w_sb[1][:, 2 * D:3 * D + 3],
                            wbig[128:256, 2 * D:3 * D + 3])

        sel_sin = trigB_sb[:, S:S + 128]
        sel_cos = trigB_sb[:, S + 128:S + 256]

        def wslice(idx, cc, col0, ncol):
            return w_sb[cc][:, idx * D + col0: idx * D + col0 + ncol]

        def bias_ap(idx, cc):
            return w_sb[cc][:, 3 * D + idx: 3 * D + idx + 1]

        # ---- PE clock ramp: F=512 garbage matmuls before inputs land ----
        for wi in range(4):
            warm_ps = pp.tile([128, SC], F32, tag="trigp", bufs=2,
                              name=f"warm_ps{wi}")
            nc.tensor.matmul(warm_ps[:], scratch[:, 0:128], scratch[:],
                             start=True, stop=True, skip_group_check=True)

        # ---- trig tables via PE broadcast (rank-2): [128,1024] sin | cos ----
        trig_sb = cpool.tile([128, 2 * S], BF16, tag="trig", name="trig_sb")
        sin_sb = trig_sb[:, 0:S]
        cos_sb = trig_sb[:, S:2 * S]
        for half in range(2):   # sin halves -> ACT evict
            ps = pp.tile([128, SC], F32, tag="trigp", bufs=2,
                         name=f"sin_ps{half}")
            nc.tensor.matmul(ps[:], sel_sin,
                             trigB_sb[:, half * SC:(half + 1) * SC],
                             start=True, stop=True)
            nc.scalar.activation(sin_sb[:, half * SC:(half + 1) * SC], ps[:],
                                 mybir.ActivationFunctionType.Copy)
        for half in range(2):   # cos halves -> DVE evict
            ps = pp.tile([128, SC], F32, tag="trigp", bufs=2,
                         name=f"cos_ps{half}")
            nc.tensor.matmul(ps[:], sel_cos,
                             trigB_sb[:, half * SC:(half + 1) * SC],
                             start=True, stop=True)
            nc.vector.tensor_scalar_mul(
                cos_sb[:, half * SC:(half + 1) * SC], ps[:], 1.0)

        def act2(tag, width=S, dtype=BF16):
            return [apool.tile([128, width], dtype, tag=f"{tag}{cc}",
                               name=f"{tag}{cc}") for cc in range(2)]

        qT = act2("qT")
        vT = act2("vT")
        rqT = act2("rqT")
        rvT = act2("rvT")
        rq_nat = [apool.tile([128, 4 * D], BF16, tag=f"rqn{sc}", name=f"rqn{sc}")
                  for sc in range(NS)]
        rv_nat = [apool.tile([128, 4 * D], BF16, tag=f"rvn{sc}", name=f"rvn{sc}")
                  for sc in range(NS)]

        # ---- projections; PE order: q0, q1, [mask MM], v0, v1 ----
        q_ps = [[None, None] for _ in range(NS)]
        v_ps = [[None, None] for _ in range(NS)]

        def proj_chunk(widx, sc, dst_ps):
            for ac in range(2):
                ps = pp.tile([128, SC], F32, tag="mm", bufs=4,
                             name=f"proj_ps_{widx}_{sc}_{ac}")
                for dc in range(2):
                    nc.tensor.matmul(
                        ps[:],
                        wslice(widx, dc, ac * 128, 128),
                        xT_sb[dc][:, sc * SC:(sc + 1) * SC],
                        start=(dc == 0), stop=(dc == 1),
                    )
                dst_ps[ac] = ps

        proj_chunk(0, 0, q_ps[0])
        proj_chunk(0, 1, q_ps[1])

        # mask via rank-16 MM (same [128,256] tile serves both Gram halves)
        mask_sb = cpool.tile([128, D], BF16, tag="mask", name="mask_sb")
        mask_ps = pp.tile([128, D], F32, tag="sm", bufs=2, name="mask_ps")
        nc.tensor.matmul(mask_ps[:], maskAB_sb[:, 0:128], maskAB_sb[:, 128:384],
                         start=True, stop=True)

        proj_chunk(1, 0, v_ps[0])
        proj_chunk(1, 1, v_ps[1])

        # ---- ACT evict stream ----
        def evict(ps, bidx, dstT, ac, sc):
            nc.scalar.activation(
                dstT[ac][:, sc * SC:(sc + 1) * SC], ps[:],
                mybir.ActivationFunctionType.Identity, bias=bias_ap(bidx, ac))

        evict(q_ps[0][0], 0, qT, 0, 0)
        evict(q_ps[0][1], 0, qT, 1, 0)
        evict(q_ps[1][0], 0, qT, 0, 1)
        evict(q_ps[1][1], 0, qT, 1, 1)
        for sc in range(NS):
            evict(v_ps[sc][0], 1, vT, 0, sc)
            evict(v_ps[sc][1], 1, vT, 1, sc)
        # mask evict late on ACT (needed only at BDT time)
        nc.scalar.activation(mask_sb[:], mask_ps[:],
                             mybir.ActivationFunctionType.Copy)

        # ---- rope (all DVE) + per-chunk transposes ----
        def rope_chunk(srcT, dstT, sc):
            sl = slice(sc * SC, (sc + 1) * SC)
            E, O = srcT[0][:, sl], srcT[1][:, sl]
            ssl = sin_sb[:, sl]
            csl = cos_sb[:, sl]
            m1 = opool.tile([128, SC], BF16, tag="m1", bufs=2)
            m2 = opool.tile([128, SC], BF16, tag="m2", bufs=2)
            m3 = opool.tile([128, SC], BF16, tag="m3", bufs=2)
            m4 = opool.tile([128, SC], BF16, tag="m4", bufs=2)
            nc.vector.tensor_tensor(m1[:], E, ssl, mybir.AluOpType.mult)
            nc.vector.tensor_tensor(m3[:], E, csl, mybir.AluOpType.mult)
            nc.vector.tensor_tensor(m2[:], O, csl, mybir.AluOpType.mult)
            nc.vector.tensor_tensor(m4[:], O, ssl, mybir.AluOpType.mult)
            nc.vector.tensor_tensor(dstT[0][:, sl], m1[:], m2[:],
                                    mybir.AluOpType.subtract)
            nc.vector.tensor_tensor(dstT[1][:, sl], m3[:], m4[:],
                                    mybir.AluOpType.add)

        def transpose_chunk(srcT, nat, sc, engines):
            nat3 = nat[:].rearrange("p (st c) -> p st c", c=D)
            for cc in range(2):
                engines[cc].dma_start(
                    nat3[:, :, cc * 128:(cc + 1) * 128],
                    srcT[cc][:, sc * SC:(sc + 1) * SC], transpose=True)

        rope_chunk(qT, rqT, 0)
        transpose_chunk(rqT, rq_nat[0], 0, [nc.sync, nc.sync])
        rope_chunk(qT, rqT, 1)
        transpose_chunk(rqT, rq_nat[1], 1, [nc.sync, nc.sync])
        rope_chunk(vT, rvT, 0)
        transpose_chunk(rvT, rv_nat[0], 0, [nc.sync, nc.scalar])
        rope_chunk(vT, rvT, 1)
        transpose_chunk(rvT, rv_nat[1], 1, [nc.sync, nc.scalar])

        # ---- Gram (PE, accumulated; ec slice depends on one transpose) ----
        gram_ps = [pp.tile([128, D], F32, tag="sm", bufs=2, name=f"gram_ps{ec}")
                   for ec in range(2)]
        for sc in range(NS):
            for ec in range(2):
                for stl in range(4):
                    st = sc * 4 + stl
                    nc.tensor.matmul(
                        gram_ps[ec][:],
                        rv_nat[sc][:, stl * D + ec * 128: stl * D + (ec + 1) * 128],
                        rq_nat[sc][:, stl * D: (stl + 1) * D],
                        start=(st == 0), stop=(st == 7),
                        skip_group_check=True,
                    )

        # ---- mask -> BDT (DVE) ----
        bdt = act2("bdt", width=D)
        for ec in range(2):
            nc.vector.tensor_tensor(
                bdt[ec][:], gram_ps[ec][:], mask_sb[:],
                mybir.AluOpType.mult)

        # ---- W2[f, o] = sum_e BDT[e, f] wot[e, o] / 4 (evict on ACT) ----
        w2 = act2("w2", width=D)
        for fc in range(2):
            ps = pp.tile([128, D], F32, tag="sm", bufs=2, name=f"w2_ps{fc}")
            for ec in range(2):
                nc.tensor.matmul(
                    ps[:],
                    bdt[ec][:, fc * 128:(fc + 1) * 128],
                    wslice(2, ec, 0, D),
                    start=(ec == 0), stop=(ec == 1),
                )
            nc.scalar.activation(w2[fc][:], ps[:],
                                 mybir.ActivationFunctionType.Copy, scale=0.25)

        # ---- final: outT[o, s] = sum_f W2[f, o] rqT[f, s] + bo ----
        for oc in range(2):
            for sc in range(NS):
                ps = pp.tile([128, SC], F32, tag="mm", bufs=4,
                             name=f"fin_ps{oc}{sc}")
                for fc in range(2):
                    nc.tensor.matmul(
                        ps[:],
                        w2[fc][:, oc * 128:(oc + 1) * 128],
                        rqT[fc][:, sc * SC:(sc + 1) * SC],
                        start=(fc == 0), stop=(fc == 1),
                    )
                ot = opool.tile([128, SC], BF16, tag="out_sb", bufs=4,
                                name=f"out_sb{oc}{sc}")
                nc.scalar.activation(
                    ot[:], ps[:],
                    mybir.ActivationFunctionType.Identity,
                    bias=bias_ap(2, oc),
                )
                eng = nc.sync if (oc + sc) % 2 == 0 else nc.scalar
                eng.dma_start(
                    outT[oc * 128:(oc + 1) * 128, sc * SC:(sc + 1) * SC], ot[:])


_NC_CACHE = None


def _get_nc():
    global _NC_CACHE
    if _NC_CACHE is None:
        _NC_CACHE = build_kernel()
    return _NC_CACHE


def make_in_maps(x, wq_w, wq_b, wv_w, wv_b, wo_w, wo_b):
    trigB, maskAB = _host_tables()
    wq_p = np.ascontiguousarray(wq_w[PERM].T).astype(ml_dtypes.bfloat16)   # [d, a]
    wv_p = np.ascontiguousarray(wv_w[PERM].T).astype(ml_dtypes.bfloat16)
    wo_p = np.ascontiguousarray(wo_w[:, PERM].T).astype(ml_dtypes.bfloat16)  # [a(e), o]
    bias3 = np.stack([wq_b[PERM], wv_b[PERM], wo_b], axis=1).astype(ml_dtypes.bfloat16)
    wbig = np.ascontiguousarray(
        np.concatenate([wq_p, wv_p, wo_p, bias3], axis=1))
    in_maps = []
    for b in range(B):
        in_maps.append({
            "xT": np.ascontiguousarray(x[b].T).astype(ml_dtypes.bfloat16),
            "wbig": wbig, "trigB": np.ascontiguousarray(trigB),
            "maskAB": np.ascontiguousarray(maskAB),
        })
    return in_maps


TRACE = False
RUN_KWARGS = {}
LAST_RESULT = None


def kernel(x, wq_w, wq_b, wk_w, wk_b, wv_w, wv_b, wo_w, wo_b):
    global LAST_RESULT
    x = np.asarray(x, dtype=np.float32)
    in_maps = make_in_maps(x, np.asarray(wq_w, np.float32), np.asarray(wq_b, np.float32),
                           np.asarray(wv_w, np.float32), np.asarray(wv_b, np.float32),
                           np.asarray(wo_w, np.float32), np.asarray(wo_b, np.float32))
    nc = _get_nc()
    res = run_bass_kernel_spmd(nc, in_maps, core_ids=list(range(N_CORES)),
                               trace=TRACE, **RUN_KWARGS)
    LAST_RESULT = res
    outs = [np.ascontiguousarray(res.results[b]["outT"].T) for b in range(B)]
    return np.stack(outs).astype(np.float32)


# revision 16
# speedup vs baseline: 1.0138x; 1.0046x over previous
"""Trainium2 Bass kernel for nn_Attention_77824807403911 (sparse_attention).

Math (per batch element, no softmax => associativity):
    q = x @ Wq^T + bq ; v = x @ Wv^T + bv          [1024, 256]
    rq = rope(q) ; rv = rope(v)
    per head h (16 heads, hd=16):  att_h = rq_h @ (rq_h^T @ rv_h) / 4
    out = att @ Wo^T + bo

Instead of the 1024x1024 score matrix we compute the 16x16 Gram per head
(64x fewer flops), realized as a full 256x256 Gram masked to the
block-diagonal, folded with Wo into a single per-batch [256,256] weight:
    F[e,f]  = sum_s rv[s,e] rq[s,f]       (Gram, transposed blocks)
    BDT     = F .* blockmask
    W2[f,o] = sum_e BDT[e,f] * Wo[o,e] / 4
    outT    = W2^T @ rqT + bo             ([256, 1024])

Sharding: data-parallel over batch, 1 element per core, no collectives.

Schedule (v4):
- input cut to ~0.9MB: trig tables and the block mask are generated
  on-chip via tiny PE broadcast matmuls (rank-2 / rank-16);
- per-ring DMA order puts the first-needed bytes (wq, first x chunks)
  at the ring heads;
- PE clock-ramp warmups sized to end when the first x chunk lands;
- s-chunked (512) pipeline proj (PE) -> evict+bias (ACT) -> rope (all
  DVE; GpSimd poisons concurrent DVE ops via the shared SBUF port) ->
  xbar transpose (Sync/Scalar) -> Gram accumulate (PE, per-slice deps);
- bf16 output on both rings (host upcasts).
"""

import numpy as np
import ml_dtypes

import concourse.bass as bass
import concourse.bacc as bacc
import concourse.tile as tile
from concourse import mybir
from concourse.bass_utils import run_bass_kernel_spmd

B, S, D, H, HD = 8, 1024, 256, 16, 16
N_CORES = 8
BF16 = mybir.dt.bfloat16
F32 = mybir.dt.float32

# channel permutation: [evens of pairs 0-63 (theta=1), evens of pairs 64-127
# (theta=1e-4), odds of pairs 0-63, odds of pairs 64-127]
PERM = np.concatenate(
    [np.arange(0, 128, 2), np.arange(128, 256, 2),
     np.arange(1, 128, 2), np.arange(129, 256, 2)]
)


def _host_tables():
    s = np.arange(S, dtype=np.float64) + 1.0
    trig = np.stack([
        np.sin(s), np.cos(s), np.sin(1e-4 * s), np.cos(1e-4 * s)
    ])                                      # [4, 1024]
    sel_sin = np.zeros((4, 128))
    sel_cos = np.zeros((4, 128))
    sel_sin[0, 0:64] = 1.0
    sel_sin[2, 64:128] = 1.0
    sel_cos[1, 0:64] = 1.0
    sel_cos[3, 64:128] = 1.0
    trigB = np.concatenate([trig, sel_sin, sel_cos],
                           axis=1).astype(ml_dtypes.bfloat16)  # [4, 1280]
    # mask = A^T B, rank 16: A[h,p] = (p//8 == h), B[h,c] = ((c%128)//8 == h)
    h = np.arange(16)
    p = np.arange(128)
    c = np.arange(256)
    A = (p[None, :] // 8 == h[:, None]).astype(np.float64)       # [16,128]
    Bm = ((c[None, :] % 128) // 8 == h[:, None]).astype(np.float64)  # [16,256]
    maskAB = np.concatenate([A, Bm], axis=1).astype(ml_dtypes.bfloat16)  # [16,384]
    return trigB, maskAB


def build_kernel():
    nc = bacc.Bacc()
    xT = nc.declare_dram_parameter("xT", [D, S], BF16, isOutput=False)
    # wbig columns: [wqt | wvt | wot], each [256, 256]; then 3 bias cols
    wbig = nc.declare_dram_parameter("wbig", [D, 3 * D + 3], BF16, isOutput=False)
    # trigB rows: sin(s), cos(s), sin(1e-4 s), cos(1e-4 s), s=1..1024, then
    # sel_sin [4,128], sel_cos [4,128]
    trigB = nc.declare_dram_parameter("trigB", [4, S + 256], BF16, isOutput=False)
    maskAB = nc.declare_dram_parameter("maskAB", [16, 384], BF16, isOutput=False)
    outT = nc.declare_dram_parameter("outT", [D, S], BF16, isOutput=True)

    with tile.TileContext(nc) as tc:
        _body(tc, xT, wbig, trigB, maskAB, outT)
    nc.compile()
    return nc


def _body(tc, xT, wbig, trigB, maskAB, outT):
    nc = tc.nc
    NS = 2          # s chunks of 512
    SC = S // NS    # 512

    with (
        tc.tile_pool(name="const", bufs=1) as cpool,
        tc.tile_pool(name="acts", bufs=1) as apool,
        tc.tile_pool(name="psum", bufs=4, space="PSUM") as pp,
        tc.tile_pool(name="outp", bufs=4) as opool,
    ):
        scratch = cpool.tile([128, 512], BF16, tag="scratch", name="scratch")
        nc.gpsimd.memset(scratch[:], 0.25)

        # ---- input DMAs: ring heads carry first-needed bytes ----
        trigB_sb = cpool.tile([4, S + 256], BF16, tag="trigB", name="trigB_sb")
        maskAB_sb = cpool.tile([16, 384], BF16, tag="maskAB", name="maskAB_sb")
        w_sb = [cpool.tile([128, 3 * D + 3], BF16, tag=f"wbig{cc}",
                           name=f"wbig{cc}") for cc in range(2)]
        xT_sb = [cpool.tile([128, S], BF16, tag=f"xT{cc}", name=f"xT{cc}")
                 for cc in range(2)]
        # sync ring: trigB, wq0, x00, x01, wv0, (wo0+bias)
        nc.sync.dma_start(trigB_sb[:], trigB[:])
        nc.sync.dma_start(w_sb[0][:, 0:D], wbig[0:128, 0:D])
        nc.sync.dma_start(xT_sb[0][:, 0:SC], xT[0:128, 0:SC])
        nc.sync.dma_start(xT_sb[0][:, SC:S], xT[0:128, SC:S])
        nc.sync.dma_start(w_sb[0][:, D:2 * D], wbig[0:128, D:2 * D])
        nc.sync.dma_start(w_sb[0][:, 2 * D:3 * D + 3], wbig[0:128, 2 * D:3 * D + 3])
        # scalar ring: maskAB, wq1, x10, x11, wv1, (wo1+bias)
        nc.scalar.dma_start(maskAB_sb[:], maskAB[:])
        nc.scalar.dma_start(w_sb[1][:, 0:D], wbig[128:256, 0:D])
        nc.scalar.dma_start(xT_sb[1][:, 0:SC], xT[128:256, 0:SC])
        nc.scalar.dma_start(xT_sb[1][:, SC:S], xT[128:256, SC:S])
        nc.scalar.dma_start(w_sb[1][:, D:2 * D], wbig[128:256, D:2 * D])
        nc.scalar.dma_start(w_sb[1][:, 2 * D:3 * D + 3],
                            wbig[128:256, 2 * D:3 * D + 3])

        sel_sin = trigB_sb[:, S:S + 128]
        sel_cos = trigB_sb[:, S + 128:S + 256]

        def wslice(idx, cc, col0, ncol):
            return w_sb[cc][:, idx * D + col0: idx * D + col0 + ncol]

        def bias_ap(idx, cc):
            return w_sb[cc][:, 3 * D + idx: 3 * D + idx + 1]

        # ---- PE clock ramp: F=512 garbage matmuls before inputs land ----
        for wi in range(3):
            warm_ps = pp.tile([128, SC], F32, tag="trigp", bufs=2,
                              name=f"warm_ps{wi}")
            nc.tensor.matmul(warm_ps[:], scratch[:, 0:128], scratch[:],
                             start=True, stop=True, skip_group_check=True)

        # ---- trig tables via PE broadcast (rank-2): [128,1024] sin | cos ----
        # high priority: the whole chain must finish before rope needs it
        trig_sb = cpool.tile([128, 2 * S], BF16, tag="trig", name="trig_sb")
        sin_sb = trig_sb[:, 0:S]
        cos_sb = trig_sb[:, S:2 * S]
        cos_ps = [None, None]
        with tc.high_priority():
            for half in range(2):   # sin halves -> ACT evict
                ps = pp.tile([128, SC], F32, tag="trigp", bufs=2,
                             name=f"sin_ps{half}")
                nc.tensor.matmul(ps[:], sel_sin,
                                 trigB_sb[:, half * SC:(half + 1) * SC],
                                 start=True, stop=True)
                nc.scalar.activation(sin_sb[:, half * SC:(half + 1) * SC],
                                     ps[:], mybir.ActivationFunctionType.Copy)
            for half in range(2):   # cos MMs now; DVE evicts placed later
                ps = pp.tile([128, SC], F32, tag="trigp", bufs=2,
                             name=f"cos_ps{half}")
                nc.tensor.matmul(ps[:], sel_cos,
                                 trigB_sb[:, half * SC:(half + 1) * SC],
                                 start=True, stop=True)
                cos_ps[half] = ps
            nc.vector.tensor_scalar_mul(cos_sb[:, 0:SC], cos_ps[0][:], 1.0)

        def act2(tag, width=S, dtype=BF16):
            return [apool.tile([128, width], dtype, tag=f"{tag}{cc}",
                               name=f"{tag}{cc}") for cc in range(2)]

        qT = act2("qT")
        vT = act2("vT")
        rqT = act2("rqT")
        rvT = act2("rvT")
        rq_nat = [apool.tile([128, 4 * D], BF16, tag=f"rqn{sc}", name=f"rqn{sc}")
                  for sc in range(NS)]
        rv_nat = [apool.tile([128, 4 * D], BF16, tag=f"rvn{sc}", name=f"rvn{sc}")
                  for sc in range(NS)]

        # ---- projections; PE order: q0, q1, [mask MM], v0, v1 ----
        q_ps = [[None, None] for _ in range(NS)]
        v_ps = [[None, None] for _ in range(NS)]

        def proj_chunk(widx, sc, dst_ps):
            for ac in range(2):
                ps = pp.tile([128, SC], F32, tag="mm", bufs=4,
                             name=f"proj_ps_{widx}_{sc}_{ac}")
                for dc in range(2):
                    nc.tensor.matmul(
                        ps[:],
                        wslice(widx, dc, ac * 128, 128),
                        xT_sb[dc][:, sc * SC:(sc + 1) * SC],
                        start=(dc == 0), stop=(dc == 1),
                    )
                dst_ps[ac] = ps

        # mask via rank-16 MM (same [128,256] tile serves both Gram halves)
        mask_sb = cpool.tile([128, D], BF16, tag="mask", name="mask_sb")
        mask_ps = pp.tile([128, D], F32, tag="sm", bufs=2, name="mask_ps")
        nc.tensor.matmul(mask_ps[:], maskAB_sb[:, 0:128], maskAB_sb[:, 128:384],
                         start=True, stop=True)

        proj_chunk(0, 0, q_ps[0])
        proj_chunk(0, 1, q_ps[1])
        proj_chunk(1, 0, v_ps[0])
        proj_chunk(1, 1, v_ps[1])

        # ---- ACT evict stream ----
        def evict(ps, bidx, dstT, ac, sc):
            nc.scalar.activation(
                dstT[ac][:, sc * SC:(sc + 1) * SC], ps[:],
                mybir.ActivationFunctionType.Identity, bias=bias_ap(bidx, ac))

        evict(q_ps[0][0], 0, qT, 0, 0)
        evict(q_ps[0][1], 0, qT, 1, 0)
        evict(q_ps[1][0], 0, qT, 0, 1)
        evict(q_ps[1][1], 0, qT, 1, 1)
        for sc in range(NS):
            evict(v_ps[sc][0], 1, vT, 0, sc)
            evict(v_ps[sc][1], 1, vT, 1, sc)
        # mask evict late on ACT (needed only at BDT time)
        nc.scalar.activation(mask_sb[:], mask_ps[:],
                             mybir.ActivationFunctionType.Copy)

        # ---- rope (all DVE) + per-chunk transposes ----
        def rope_chunk(srcT, dstT, sc):
            sl = slice(sc * SC, (sc + 1) * SC)
            E, O = srcT[0][:, sl], srcT[1][:, sl]
            ssl = sin_sb[:, sl]
            csl = cos_sb[:, sl]
            m1 = opool.tile([128, SC], BF16, tag="m1", bufs=2)
            m2 = opool.tile([128, SC], BF16, tag="m2", bufs=2)
            m3 = opool.tile([128, SC], BF16, tag="m3", bufs=2)
            m4 = opool.tile([128, SC], BF16, tag="m4", bufs=2)
            nc.vector.tensor_tensor(m1[:], E, ssl, mybir.AluOpType.mult)
            nc.vector.tensor_tensor(m3[:], E, csl, mybir.AluOpType.mult)
            nc.vector.tensor_tensor(m2[:], O, csl, mybir.AluOpType.mult)
            nc.vector.tensor_tensor(m4[:], O, ssl, mybir.AluOpType.mult)
            nc.vector.tensor_tensor(dstT[0][:, sl], m1[:], m2[:],
                                    mybir.AluOpType.subtract)
            nc.vector.tensor_tensor(dstT[1][:, sl], m3[:], m4[:],
                                    mybir.AluOpType.add)

        def transpose_chunk(srcT, nat, sc, engines):
            nat3 = nat[:].rearrange("p (st c) -> p st c", c=D)
            for cc in range(2):
                engines[cc].dma_start(
                    nat3[:, :, cc * 128:(cc + 1) * 128],
                    srcT[cc][:, sc * SC:(sc + 1) * SC], transpose=True)

        rope_chunk(qT, rqT, 0)
        transpose_chunk(rqT, rq_nat[0], 0, [nc.sync, nc.sync])
        # cos sc1 evict slotted into the DVE stream before rope q1 needs it
        nc.vector.tensor_scalar_mul(cos_sb[:, SC:S], cos_ps[1][:], 1.0)
        rope_chunk(qT, rqT, 1)
        transpose_chunk(rqT, rq_nat[1], 1, [nc.sync, nc.sync])
        rope_chunk(vT, rvT, 0)
        transpose_chunk(rvT, rv_nat[0], 0, [nc.scalar, nc.scalar])
        rope_chunk(vT, rvT, 1)
        transpose_chunk(rvT, rv_nat[1], 1, [nc.sync, nc.scalar])

        # ---- Gram (PE, accumulated; ec slice depends on one transpose) ----
        gram_ps = [pp.tile([128, D], F32, tag="sm", bufs=2, name=f"gram_ps{ec}")
                   for ec in range(2)]
        for sc in range(NS):
            for ec in range(2):
                for stl in range(4):
                    st = sc * 4 + stl
                    nc.tensor.matmul(
                        gram_ps[ec][:],
                        rv_nat[sc][:, stl * D + ec * 128: stl * D + (ec + 1) * 128],
                        rq_nat[sc][:, stl * D: (stl + 1) * D],
                        start=(st == 0), stop=(st == 7),
                        skip_group_check=True,
                    )

        # ---- mask -> BDT (DVE) ----
        bdt = act2("bdt", width=D)
        for ec in range(2):
            nc.vector.tensor_tensor(
                bdt[ec][:], gram_ps[ec][:], mask_sb[:],
                mybir.AluOpType.mult)

        # ---- W2[f, o] = sum_e BDT[e, f] wot[e, o] / 4 (evict on ACT) ----
        w2 = act2("w2", width=D)
        for fc in range(2):
            ps = pp.tile([128, D], F32, tag="sm", bufs=2, name=f"w2_ps{fc}")
            for ec in range(2):
                nc.tensor.matmul(
                    ps[:],
                    bdt[ec][:, fc * 128:(fc + 1) * 128],
                    wslice(2, ec, 0, D),
                    start=(ec == 0), stop=(ec == 1),
                )
            nc.scalar.activation(w2[fc][:], ps[:],
                                 mybir.ActivationFunctionType.Copy, scale=0.25)

        # ---- final: outT[o, s] = sum_f W2[f, o] rqT[f, s] + bo ----
        for oc in range(2):
            for sc in range(NS):
                ps = pp.tile([128, SC], F32, tag="mm", bufs=4,
                             name=f"fin_ps{oc}{sc}")
                for fc in range(2):
                    nc.tensor.matmul(
                        ps[:],
                        w2[fc][:, oc * 128:(oc + 1) * 128],
                        rqT[fc][:, sc * SC:(sc + 1) * SC],
                        start=(fc == 0), stop=(fc == 1),
                    )
                ot = opool.tile([128, SC], BF16, tag="out_sb", bufs=4,
                                name=f"out_sb{oc}{sc}")
                nc.scalar.activation(
                    ot[:], ps[:],
                    mybir.ActivationFunctionType.Identity,
                    bias=bias_ap(2, oc),
                )
                eng = nc.sync if (oc + sc) % 2 == 0 else nc.scalar
                eng.dma_start(
                    outT[oc * 128:(oc + 1) * 128, sc * SC:(sc + 1) * SC], ot[:])


_NC_CACHE = None


def _get_nc():
    global _NC_CACHE
    if _NC_CACHE is None:
        _NC_CACHE = build_kernel()
    return _NC_CACHE


def make_in_maps(x, wq_w, wq_b, wv_w, wv_b, wo_w, wo_b):
    trigB, maskAB = _host_tables()
    wq_p = np.ascontiguousarray(wq_w[PERM].T).astype(ml_dtypes.bfloat16)   # [d, a]
    wv_p = np.ascontiguousarray(wv_w[PERM].T).astype(ml_dtypes.bfloat16)
    wo_p = np.ascontiguousarray(wo_w[:, PERM].T).astype(ml_dtypes.bfloat16)  # [a(e), o]
    bias3 = np.stack([wq_b[PERM], wv_b[PERM], wo_b], axis=1).astype(ml_dtypes.bfloat16)
    wbig = np.ascontiguousarray(
        np.concatenate([wq_p, wv_p, wo_p, bias3], axis=1))
    in_maps = []
    for b in range(B):
        in_maps.append({
            "xT": np.ascontiguousarray(x[b].T).astype(ml_dtypes.bfloat16),
            "wbig": wbig, "trigB": np.ascontiguousarray(trigB),
            "maskAB": np.ascontiguousarray(maskAB),
        })
    return in_maps


TRACE = False
RUN_KWARGS = {}
LAST_RESULT = None


def kernel(x, wq_w, wq_b, wk_w, wk_b, wv_w, wv_b, wo_w, wo_b):
    global LAST_RESULT
    x = np.asarray(x, dtype=np.float32)
    in_maps = make_in_maps(x, np.asarray(wq_w, np.float32), np.asarray(wq_b, np.float32),
                           np.asarray(wv_w, np.float32), np.asarray(wv_b, np.float32),
                           np.asarray(wo_w, np.float32), np.asarray(wo_b, np.float32))
    nc = _get_nc()
    res = run_bass_kernel_spmd(nc, in_maps, core_ids=list(range(N_CORES)),
                               trace=TRACE, **RUN_KWARGS)
    LAST_RESULT = res
    outs = [np.ascontiguousarray(res.results[b]["outT"].T) for b in range(B)]
    return np.stack(outs).astype(np.float32)


# revision 17
# speedup vs baseline: 1.1079x; 1.0928x over previous
"""Trainium2 Bass kernel for nn_Attention_77824807403911 (sparse_attention).

Math (per batch element, no softmax => associativity):
    q = x @ Wq^T + bq ; v = x @ Wv^T + bv          [1024, 256]
    rq = rope(q) ; rv = rope(v)
    per head h (16 heads, hd=16):  att_h = rq_h @ (rq_h^T @ rv_h) / 4
    out = att @ Wo^T + bo

Instead of the 1024x1024 score matrix we compute the 16x16 Gram per head
(64x fewer flops), realized as a full 256x256 Gram masked to the
block-diagonal, folded with Wo into a single per-batch [256,256] weight:
    F[e,f]  = sum_s rv[s,e] rq[s,f]       (Gram, transposed blocks)
    BDT     = F .* blockmask
    W2[f,o] = sum_e BDT[e,f] * Wo[o,e] / 4
    outT    = W2^T @ rqT + bo             ([256, 1024])

Sharding: data-parallel over batch, 1 element per core, no collectives.

v6 deltas over the original schedule:
- input DMA ring order puts weights + x first, trig halves last (the
  rings drain roughly FIFO; rope needs trig only ~3us after the first
  projection matmul);
- x split into 512-col halves so the second projection chunk isn't
  gated on the whole 256KB row block;
- output stored bf16 (host upcasts) halving output traffic, with the
  four output DMAs alternating across both HWDGE rings;
- eviction order ac-major so full-width rope starts after 2 evicts.
"""

import numpy as np
import ml_dtypes

import concourse.bass as bass
import concourse.bacc as bacc
import concourse.tile as tile
from concourse import mybir
from concourse.bass_utils import run_bass_kernel_spmd

B, S, D, H, HD = 8, 1024, 256, 16, 16
N_CORES = 8
BF16 = mybir.dt.bfloat16
F32 = mybir.dt.float32

PERM = np.concatenate(
    [np.arange(0, 128, 2), np.arange(128, 256, 2),
     np.arange(1, 128, 2), np.arange(129, 256, 2)]
)


def _host_tables():
    p = np.arange(128)
    theta = np.where(p < 64, 1.0, 1e-4)
    s = np.arange(S, dtype=np.float64) + 1.0
    ang = theta[:, None] * s[None, :]
    sin = np.sin(ang).astype(ml_dtypes.bfloat16)
    cos = np.cos(ang).astype(ml_dtypes.bfloat16)
    a = np.arange(256)
    headp = (a % 128) // 8
    mask = (headp[:, None] == headp[None, :]).astype(ml_dtypes.bfloat16)
    return sin, cos, mask


def build_kernel():
    nc = bacc.Bacc()
    xT = nc.declare_dram_parameter("xT", [D, S], BF16, isOutput=False)
    # wbig columns: [wqt | wvt | wot | mask], each [256, 256], + 3 bias cols
    wbig = nc.declare_dram_parameter("wbig", [D, 4 * D + 3], BF16, isOutput=False)
    # trig columns: [sin | cos], each [128, 1024]
    trig = nc.declare_dram_parameter("trig", [128, 2 * S], BF16, isOutput=False)
    outT = nc.declare_dram_parameter("outT", [D, S], BF16, isOutput=True)

    with tile.TileContext(nc) as tc:
        _body(tc, xT, wbig, trig, outT)
    nc.compile()
    return nc


def _body(tc, xT, wbig, trig, outT):
    nc = tc.nc
    NS = 2          # s chunks of 512 for matmul streaming
    SC = S // NS    # 512

    with (
        tc.tile_pool(name="const", bufs=1) as cpool,
        tc.tile_pool(name="acts", bufs=1) as apool,
        tc.tile_pool(name="psum", bufs=4, space="PSUM") as pp,
        tc.tile_pool(name="outp", bufs=4) as opool,
    ):
        # PE warm-up: garbage matmuls release the HAM clock gate while the
        # real inputs stream in. scratch is memset early in the preamble.
        scratch = cpool.tile([128, 512], BF16, tag="scratch", name="scratch")
        nc.gpsimd.memset(scratch[:], 0.25)
        warm_ps = pp.tile([128, 512], F32, tag="warm", bufs=1, name="warm_ps")
        for wi in range(8):
            nc.tensor.matmul(warm_ps[:], scratch[:, 0:128], scratch[:],
                             start=True, stop=True, skip_group_check=True)

        # ---- input DMAs; ring order: weights, x halves, trig half ----
        trig_sb = cpool.tile([128, 2 * S], BF16, tag="trig", name="trig_sb")
        sin_sb = trig_sb[:, 0:S]
        cos_sb = trig_sb[:, S:2 * S]
        xT_sb, w_sb = [], []
        for cc in range(2):
            xT_sb.append(cpool.tile([128, S], BF16, tag=f"xT{cc}", name=f"xT{cc}"))
            w_sb.append(cpool.tile([128, 4 * D + 3], BF16, tag=f"wbig{cc}",
                                   name=f"wbig{cc}"))
        nc.sync.dma_start(w_sb[0][:], wbig[0:128, :])
        nc.scalar.dma_start(w_sb[1][:], wbig[128:256, :])
        nc.sync.dma_start(xT_sb[0][:, 0:SC], xT[0:128, 0:SC])
        nc.scalar.dma_start(xT_sb[1][:, 0:SC], xT[128:256, 0:SC])
        nc.sync.dma_start(xT_sb[0][:, SC:S], xT[0:128, SC:S])
        nc.scalar.dma_start(xT_sb[1][:, SC:S], xT[128:256, SC:S])
        nc.scalar.dma_start(sin_sb, trig[:, 0:S])
        nc.sync.dma_start(cos_sb, trig[:, S:2 * S])

        def wslice(idx, cc, col0, ncol):
            return w_sb[cc][:, idx * D + col0: idx * D + col0 + ncol]

        def bias_ap(idx, cc):
            return w_sb[cc][:, 4 * D + idx: 4 * D + idx + 1]

        def act2(tag, width=S, dtype=BF16):
            return [apool.tile([128, width], dtype, tag=f"{tag}{cc}",
                               name=f"{tag}{cc}") for cc in range(2)]

        qT = act2("qT")
        vT = act2("vT")
        rqT = act2("rqT")
        rvT = act2("rvT")
        rq_nat = apool.tile([128, 8 * D], BF16, tag="rq_nat")
        rv_nat = apool.tile([128, 8 * D], BF16, tag="rv_nat")

        # ---- projections: tT[a, s] = sum_d w[d, a] x[d, s] ----
        ps_map = {}

        def project(widx, dst):
            for ac in range(2):
                for sc in range(NS):
                    ps = pp.tile([128, SC], F32, tag="mm", bufs=5,
                                 name=f"proj_ps{widx}{ac}{sc}")
                    for dc in range(2):
                        nc.tensor.matmul(
                            ps[:],
                            wslice(widx, dc, ac * 128, 128),
                            xT_sb[dc][:, sc * SC:(sc + 1) * SC],
                            start=(dc == 0), stop=(dc == 1),
                        )
                    ps_map[(widx, ac, sc)] = ps

        project(0, qT)   # q
        project(1, vT)   # v

        # evicts ac-major: full-width rope needs [E then O] halves
        for widx, dstT, bidx in ((0, qT, 0), (1, vT, 1)):
            for ac in range(2):
                for sc in range(NS):
                    nc.scalar.activation(
                        dstT[ac][:, sc * SC:(sc + 1) * SC],
                        ps_map[(widx, ac, sc)][:],
                        mybir.ActivationFunctionType.Identity,
                        bias=bias_ap(bidx, ac),
                    )

        # ---- rope (full-width): rE = E*sin - O*cos ; rO = E*cos + O*sin ----
        def rope(srcT, dstT, tmp_tag):
            E, O = srcT[0][:], srcT[1][:]
            t1 = opool.tile([128, S], BF16, tag=tmp_tag + "1", name=tmp_tag + "1")
            t2 = opool.tile([128, S], BF16, tag=tmp_tag + "2", name=tmp_tag + "2")
            nc.vector.tensor_tensor(t1[:], E, sin_sb, mybir.AluOpType.mult)
            nc.vector.tensor_tensor(t2[:], O, cos_sb, mybir.AluOpType.mult)
            nc.vector.tensor_tensor(dstT[0][:], t1[:], t2[:], mybir.AluOpType.subtract)
            t3 = opool.tile([128, S], BF16, tag=tmp_tag + "3", name=tmp_tag + "3")
            t4 = opool.tile([128, S], BF16, tag=tmp_tag + "4", name=tmp_tag + "4")
            nc.vector.tensor_tensor(t3[:], E, cos_sb, mybir.AluOpType.mult)
            nc.vector.tensor_tensor(t4[:], O, sin_sb, mybir.AluOpType.mult)
            nc.vector.tensor_tensor(dstT[1][:], t3[:], t4[:], mybir.AluOpType.add)

        rope(qT, rqT, "rq_tmp")
        rope(vT, rvT, "rv_tmp")

        # keep the PE clock-gate open across the rope gap
        for wi in range(8):
            srcv = vT[wi % 2]
            nc.tensor.matmul(warm_ps[:], srcv[:, 0:128], srcv[:, 0:512],
                             start=True, stop=True, skip_group_check=True)

        # ---- whole-chunk transposes to natural layout (both rings) ----
        rq_nat3 = rq_nat[:].rearrange("p (st c) -> p st c", c=D)
        rv_nat3 = rv_nat[:].rearrange("p (st c) -> p st c", c=D)
        nc.sync.dma_start(rq_nat3[:, :, 0:128], rqT[0][:], transpose=True)
        nc.scalar.dma_start(rq_nat3[:, :, 128:256], rqT[1][:], transpose=True)
        nc.sync.dma_start(rv_nat3[:, :, 0:128], rvT[0][:], transpose=True)
        nc.scalar.dma_start(rv_nat3[:, :, 128:256], rvT[1][:], transpose=True)

        # ---- Gram: Hm[e, f] = sum_s rv[s, e] rq[s, f]; mask -> BDT ----
        bdt = act2("bdt", width=D)
        for ec in range(2):
            ps = pp.tile([128, D], F32, tag="sm", bufs=2, name=f"gram_ps{ec}")
            for st in range(8):
                nc.tensor.matmul(
                    ps[:],
                    rv_nat[:, st * D + ec * 128: st * D + (ec + 1) * 128],
                    rq_nat[:, st * D: (st + 1) * D],
                    start=(st == 0), stop=(st == 7),
                )
            nc.vector.tensor_tensor(
                bdt[ec][:], ps[:], wslice(3, ec, 0, D), mybir.AluOpType.mult)

        # ---- W2[f, o] = sum_e BDT[e, f] wot[e, o] (scaled 1/4 at evict) ----
        w2 = act2("w2", width=D)
        for fc in range(2):
            ps = pp.tile([128, D], F32, tag="sm", bufs=2, name=f"w2_ps{fc}")
            for ec in range(2):
                nc.tensor.matmul(
                    ps[:],
                    bdt[ec][:, fc * 128:(fc + 1) * 128],
                    wslice(2, ec, 0, D),
                    start=(ec == 0), stop=(ec == 1),
                )
            nc.scalar.activation(
                w2[fc][:], ps[:],
                mybir.ActivationFunctionType.Copy, scale=0.25)

        # ---- final: outT[o, s] = sum_f W2[f, o] rqT[f, s] + bo ----
        for oc in range(2):
            for sc in range(NS):
                ps = pp.tile([128, SC], F32, tag="mm", bufs=5,
                             name=f"fin_ps{oc}{sc}")
                for fc in range(2):
                    nc.tensor.matmul(
                        ps[:],
                        w2[fc][:, oc * 128:(oc + 1) * 128],
                        rqT[fc][:, sc * SC:(sc + 1) * SC],
                        start=(fc == 0), stop=(fc == 1),
                    )
                ot = opool.tile([128, SC], BF16, tag="out_sb", name=f"out_sb{oc}{sc}")
                nc.scalar.activation(
                    ot[:], ps[:],
                    mybir.ActivationFunctionType.Identity,
                    bias=bias_ap(2, oc),
                )
                eng = nc.scalar if (oc + sc) % 2 == 0 else nc.sync
                eng.dma_start(
                    outT[oc * 128:(oc + 1) * 128, sc * SC:(sc + 1) * SC], ot[:])


_NC_CACHE = None


def _get_nc():
    global _NC_CACHE
    if _NC_CACHE is None:
        _NC_CACHE = build_kernel()
    return _NC_CACHE


def make_in_maps(x, wq_w, wq_b, wv_w, wv_b, wo_w, wo_b):
    sin, cos, mask = _host_tables()
    wq_p = np.ascontiguousarray(wq_w[PERM].T).astype(ml_dtypes.bfloat16)
    wv_p = np.ascontiguousarray(wv_w[PERM].T).astype(ml_dtypes.bfloat16)
    wo_p = np.ascontiguousarray(wo_w[:, PERM].T).astype(ml_dtypes.bfloat16)
    bias3 = np.stack([wq_b[PERM], wv_b[PERM], wo_b], axis=1).astype(ml_dtypes.bfloat16)
    wbig = np.ascontiguousarray(
        np.concatenate([wq_p, wv_p, wo_p, mask, bias3], axis=1))
    trig = np.ascontiguousarray(np.concatenate([sin, cos], axis=1))
    in_maps = []
    for b in range(B):
        in_maps.append({
            "xT": np.ascontiguousarray(x[b].T).astype(ml_dtypes.bfloat16),
            "wbig": wbig, "trig": trig,
        })
    return in_maps


TRACE = False
RUN_KWARGS = {}
LAST_RESULT = None


def kernel(x, wq_w, wq_b, wk_w, wk_b, wv_w, wv_b, wo_w, wo_b):
    global LAST_RESULT
    x = np.asarray(x, dtype=np.float32)
    in_maps = make_in_maps(x, np.asarray(wq_w, np.float32), np.asarray(wq_b, np.float32),
                           np.asarray(wv_w, np.float32), np.asarray(wv_b, np.float32),
                           np.asarray(wo_w, np.float32), np.asarray(wo_b, np.float32))
    nc = _get_nc()
    res = run_bass_kernel_spmd(nc, in_maps, core_ids=list(range(N_CORES)),
                               trace=TRACE, **RUN_KWARGS)
    LAST_RESULT = res
    outs = [np.ascontiguousarray(res.results[b]["outT"].T) for b in range(B)]
    return np.stack(outs).astype(np.float32)
